# revision 48
# baseline (speedup 1.0000x reference)
"""Trainium2 Bass kernel for causal attention with xpos rotary embedding.

Reference computation (B=1, S=2048, D=2048, H=16 heads, dq=128):
    qkv = x @ w_qkv ; split into per-head q,k,v
    rope(q, scale), rope(k, 1/scale)  (xpos, first 32 dims of each head)
    causal softmax attention per head
    out = concat_heads @ w_out

Sharding: tensor-parallel over heads — each of the 8 cores gets 2 heads
(w_qkv column slice, w_out row slice), computes a full [S, D] partial of
the output projection; host sums the 8 partials (the "all-reduce").

Device kernel strategy (per core, bf16 data / f32 psum accumulate):
  One flat Tile region; phases interleaved per chunk so the scheduler can
  overlap PE (matmuls), ACT (exp), DVE (drains), Pool (rope), DMA.
  Phase A chunk c (CK=256 cols of x^T):
    qT/kT = w^T @ x^T per head (PE, bf16), rope applied in-place on the
    first 32 rows: rotate-half via host permutation matmul (PE) then
    3 tensor_tensor ops on the Pool engine (host cos/sin tables).
    v computed directly in [S, dq] layout (x-tile stationary) — no
    transposes.
  Phase B chunk (QC=512 q cols, after A covers its key span): per head,
  per ki pair: scoresT = kT_tile.T @ qT (PE) -> wide exp (ACT, sm_scale
  folded) -> diag-block mask (Pool) -> PV + ones-denominator accumulate
  (PE). Denominator reciprocal via DVE reciprocal_approx_fast; the
  normalize is folded into the PV psum->SBUF drain (DVE).
  Phase C (per q-tile, right after its B chunk): y = sum_h OT_h^T @ w_out
  (PE) -> psum drained to bf16 SBUF (DVE/ACT alternating) -> one DMA per
  q-tile row block. Host sums 8 bf16 partials in f32.
"""
import sys
sys.path.insert(0, "/opt/trn_rl_repo")
import numpy as np

S = 2048
D = 2048
NH = 16
DQ = 128
NROT = 32
SCALE_BASE = 512.0
NCORES = 8
HPC = NH // NCORES          # heads per core = 2
OCN = 3 * HPC               # 6 col tiles of 128 per core (q0,k0,q1,k1,v0,v1)
CK = 512                    # phase-A S-chunk width
NCK = S // CK
QC = 512                    # phase-B q-chunk width
NQC = S // QC
ST = S // 128               # 16 tiles
DC = 512                    # phase-C d-chunk width
SM_SCALE = float(1.0 / np.sqrt(DQ))

_CACHE = {}


def _build_program(repeat=1, bench=False):
    import concourse.bacc as bacc
    import concourse.tile as tile
    import concourse.mybir as mybir

    f32 = mybir.dt.float32
    bf16 = mybir.dt.bfloat16
    AF = mybir.ActivationFunctionType
    OP = mybir.AluOpType

    nc = bacc.Bacc("TRN2", target_bir_lowering=False, debug=False,
                   enable_asserts=False, num_devices=NCORES)

    xT_d = nc.dram_tensor("xT", [D, S], bf16, kind="ExternalInput").ap()
    wq_d = nc.dram_tensor("wq", [D, 128 * OCN], bf16, kind="ExternalInput").ap()
    wo_d = nc.dram_tensor("wo", [HPC * DQ, D], bf16, kind="ExternalInput").ap()
    tabs_d = nc.dram_tensor("tabs", [NROT, 4, S], bf16, kind="ExternalInput").ap()
    tri_d = nc.dram_tensor("tri", [128, 128], bf16, kind="ExternalInput").ap()
    pt_d = nc.dram_tensor("pt", [NROT, NROT], bf16, kind="ExternalInput").ap()
    ones_d = nc.dram_tensor("ones", [128, 128], bf16, kind="ExternalInput").ap()
    if bench:
        y_d = nc.dram_tensor("y", [S, D], bf16, kind="Internal").ap()
        ysm_d = nc.dram_tensor("ysm", [128, 64], bf16, kind="ExternalOutput").ap()
    else:
        y_d = nc.dram_tensor("y", [S, D], bf16, kind="ExternalOutput").ap()

    xT_r = xT_d.rearrange("(t p) s -> p t s", p=128)
    wq_r = wq_d.rearrange("(t p) f -> p t f", p=128)

    with tile.TileContext(nc) as tc:
      for rep in range(repeat):
        with tc.tile_pool(name="consts", bufs=1) as consts, \
             tc.tile_pool(name="persist", bufs=1) as persist, \
             tc.tile_pool(name="xtp", bufs=2) as xtp, \
             tc.tile_pool(name="rotp", bufs=2) as rotp, \
             tc.tile_pool(name="pp", bufs=3) as pp, \
             tc.tile_pool(name="recp", bufs=2) as recp, \
             tc.tile_pool(name="ysp", bufs=2) as ysp, \
             tc.tile_pool(name="psA", bufs=2, space="PSUM") as psA, \
             tc.tile_pool(name="psS", bufs=2, space="PSUM") as psS, \
             tc.tile_pool(name="psOT", bufs=1, space="PSUM") as psOT, \
             tc.tile_pool(name="psDen", bufs=1, space="PSUM") as psDen:

            # ---------------- input loads
            xt0 = xtp.tile([128, ST, CK], bf16, tag="xt")
            nc.sync.dma_start(xt0, xT_r[:, :, 0:CK])
            wq_sb = persist.tile([128, ST, 128 * OCN], bf16, tag="wq_sb")
            for t in range(ST):
                nc.sync.dma_start(wq_sb[:, t, :], wq_r[:, t, :])
            tabs = consts.tile([NROT, 4, S], bf16, tag="tabs")
            nc.sync.dma_start(tabs, tabs_d)
            pt = consts.tile([NROT, NROT], bf16, tag="pt")
            nc.sync.dma_start(pt, pt_d)
            tri = consts.tile([128, 128], bf16, tag="tri")
            nc.sync.dma_start(tri, tri_d)
            ones = consts.tile([128, 128], bf16, tag="ones")
            nc.sync.dma_start(ones, ones_d)
            wo_sb = persist.tile([128, HPC, D], bf16, tag="wo_sb")
            nc.sync.dma_start(wo_sb, wo_d.rearrange("(h p) f -> p h f", p=128))

            qT = [persist.tile([128, S], bf16, name=f"qT{h}", tag=f"qT{h}") for h in range(HPC)]
            kT = [persist.tile([128, S], bf16, name=f"kT{h}", tag=f"kT{h}") for h in range(HPC)]
            vn = [persist.tile([128, ST, 128], bf16, name=f"vn{h}", tag=f"vn{h}") for h in range(HPC)]
            OT = [persist.tile([128, S], bf16, name=f"OT{h}", tag=f"OT{h}") for h in range(HPC)]

            def emit_A_chunk(c, xt):
                cs = slice(c * CK, (c + 1) * CK)
                for oc in range(4):     # q_h0, k_h0, q_h1, k_h1
                    h, kind = divmod(oc, 2)
                    ps = psA.tile([128, CK], f32, tag="qkv")
                    for t in range(ST):
                        nc.tensor.matmul(
                            ps, wq_sb[:, t, oc * 128:(oc + 1) * 128],
                            xt[:, t, :], start=(t == 0), stop=(t == ST - 1))
                    dst = qT[h] if kind == 0 else kT[h]
                    if kind == 0:
                        nc.vector.tensor_copy(dst[:, cs], ps)
                    else:
                        nc.scalar.copy(dst[:, cs], ps)
                    # rope on rows 0:NROT, in place (rot psum rides the psA ring)
                    rps = psA.tile([128, CK], f32, tag="qkv")
                    nc.tensor.matmul(rps[0:NROT, :], pt, dst[0:NROT, cs],
                                     start=True, stop=True)
                    rot = rotp.tile([NROT, CK], bf16, tag="rotsb")
                    nc.scalar.copy(rot, rps[0:NROT, :])
                    ti = 0 if kind == 0 else 2
                    nc.gpsimd.tensor_tensor(
                        out=dst[0:NROT, cs], in0=dst[0:NROT, cs],
                        in1=tabs[:, ti, cs], op=OP.mult)
                    nc.gpsimd.tensor_tensor(
                        out=rot, in0=rot, in1=tabs[:, ti + 1, cs], op=OP.mult)
                    nc.gpsimd.tensor_tensor(
                        out=dst[0:NROT, cs], in0=dst[0:NROT, cs],
                        in1=rot, op=OP.add)
                # v directly in [s, dq] layout, both heads' cols at once
                # (shares the psA ring: same shape/tag as the qkv psum tiles)
                for si in range(CK // 128):
                    st = c * (CK // 128) + si
                    pvt = psA.tile([128, CK], f32, tag="qkv")
                    pv = pvt[:, 0:256]
                    for t in range(ST):
                        nc.tensor.matmul(
                            pv, xt[:, t, si * 128:(si + 1) * 128],
                            wq_sb[:, t, 4 * 128:6 * 128],
                            start=(t == 0), stop=(t == ST - 1))
                    nc.vector.tensor_copy(vn[0][:, st, :], pvt[:, 0:128])
                    nc.scalar.copy(vn[1][:, st, :], pvt[:, 128:256])

            def emit_B_chunk(qt0, qt1):
                # attention for q-tiles [qt0, qt1): history keys ki < qt0 at
                # full width, diagonal keys ki in [qt0, qt1) on the live
                # q-subrange only (triangle mask on the 128-wide diag block).
                w = 128 * (qt1 - qt0)
                q0 = 128 * qt0
                for h in range(HPC):
                    ot = psOT.tile([128, QC], f32, tag="ot")
                    dn = psDen.tile([128, QC], f32, tag="dn")
                    for m in range(qt1 - qt0):
                        ki = qt0 + m
                        qoff = 128 * m
                        sp = psS.tile([128, 2, QC], f32, tag="sc")
                        nc.tensor.matmul(
                            sp[:, 0, qoff:w], kT[h][:, ki * 128:(ki + 1) * 128],
                            qT[h][:, q0 + qoff:q0 + w], start=True, stop=True)
                        p = pp.tile([128, 2, QC], bf16, tag="p")
                        nc.scalar.activation(p[:, 0, qoff:w], sp[:, 0, qoff:w],
                                             AF.Exp, scale=SM_SCALE)
                        nc.vector.tensor_tensor(
                            out=p[:, 0, qoff:qoff + 128],
                            in0=p[:, 0, qoff:qoff + 128], in1=tri, op=OP.mult)
                        last = (qt0 == 0 and m == qt1 - qt0 - 1)
                        nc.tensor.matmul(
                            ot[:, qoff:w], vn[h][:, ki, :], p[:, 0, qoff:w],
                            start=(m == 0), stop=last, skip_group_check=True)
                        nc.tensor.matmul(
                            dn[:, qoff:w], ones, p[:, 0, qoff:w],
                            start=(m == 0), stop=last, skip_group_check=True)
                    # full-width history tiles (ki < qt0), paired exp
                    for kj in range(qt0 // 2):
                        sp = psS.tile([128, 2, QC], f32, tag="sc")
                        for u in range(2):
                            ki = 2 * kj + u
                            nc.tensor.matmul(
                                sp[:, u, 0:w], kT[h][:, ki * 128:(ki + 1) * 128],
                                qT[h][:, q0:q0 + w], start=True, stop=True)
                        p = pp.tile([128, 2, QC], bf16, tag="p")
                        nc.scalar.activation(p[:, :, 0:w], sp[:, :, 0:w],
                                             AF.Exp, scale=SM_SCALE)
                        last = (2 * kj + 1 == qt0 - 1)
                        for u in range(2):
                            nc.tensor.matmul(
                                ot[:, 0:w], vn[h][:, 2 * kj + u, :], p[:, u, 0:w],
                                start=False, stop=(last and u == 1),
                                skip_group_check=True)
                            nc.tensor.matmul(
                                dn[:, 0:w], ones, p[:, u, 0:w],
                                start=False, stop=(last and u == 1),
                                skip_group_check=True)
                    rec = recp.tile([128, QC], f32, tag="rec")
                    nc.vector.reciprocal_approx_fast(rec[:, 0:w], dn[:, 0:w])
                    nc.vector.tensor_tensor(
                        out=OT[h][:, q0:q0 + w], in0=ot[:, 0:w],
                        in1=rec[:, 0:w], op=OP.mult)

            def emit_C_chunk(qt0, qt1):
                # yp psum rides the psS ring ([128, 2, QC] = two DC-wide chunks)
                for qt in range(qt0, qt1):
                    ys = ysp.tile([128, D], bf16, tag="ys")
                    for dc2 in range(D // (2 * DC)):
                        yp = psS.tile([128, 2, QC], f32, tag="sc")
                        for u in range(2):
                            ds_ = slice((2 * dc2 + u) * DC, (2 * dc2 + u + 1) * DC)
                            for h in range(HPC):
                                nc.tensor.matmul(
                                    yp[:, u, :], OT[h][:, qt * 128:(qt + 1) * 128],
                                    wo_sb[:, h, ds_],
                                    start=(h == 0), stop=(h == HPC - 1))
                            if (dc2 + u) % 2 == 0:
                                nc.vector.tensor_copy(ys[:, ds_], yp[:, u, :])
                            else:
                                nc.scalar.copy(ys[:, ds_], yp[:, u, :])
                    nc.sync.dma_start(y_d[qt * 128:(qt + 1) * 128, :], ys)
                    if bench and qt == ST - 1:
                        nc.sync.dma_start(ysm_d, ys[:, 0:64])

            # B/C sub-chunk (qt0, qt1) emitted after A chunk (qt1 // 2) - 1;
            # the final 512-wide chunk is split so most of its work can
            # start before the last A chunk lands.
            bc_after = {0: [(0, 4)], 1: [(4, 8)], 2: [(8, 12)], 3: [(12, 16)]}
            for c in range(NCK):
                if c == 0:
                    xt = xt0
                else:
                    xt = xtp.tile([128, ST, CK], bf16, tag="xt")
                    nc.sync.dma_start(xt, xT_r[:, :, c * CK:(c + 1) * CK])
                emit_A_chunk(c, xt)
                for qt0, qt1 in bc_after.get(c, []):
                    emit_B_chunk(qt0, qt1)
                    emit_C_chunk(qt0, qt1)
    nc.compile()
    return nc


def _host_tables():
    """cos/sin xpos tables, computed in fp32 mirroring the jax reference."""
    t = np.arange(S, dtype=np.float32)
    inv_freq = (1.0 / (10000.0 ** (np.arange(0, NROT, 2, dtype=np.float32) / NROT))
                ).astype(np.float32)
    freqs = t[:, None] * inv_freq[None, :]              # [S, 16]
    pos = np.concatenate([freqs, freqs], axis=-1)       # [S, 32]
    base_scale = ((np.arange(0, NROT, 2, dtype=np.float32) + 0.4 * NROT)
                  / (1.4 * NROT)).astype(np.float32)
    power = (t - S // 2) / np.float32(SCALE_BASE)
    scale = base_scale[None, :] ** power[:, None]       # [S, 16]
    scale = np.concatenate([scale, scale], axis=-1)     # [S, 32]
    cos, sin = np.cos(pos), np.sin(pos)
    rscale = (np.float32(1.0) / scale).astype(np.float32)
    tabs = np.stack([
        (cos * scale).T, (sin * scale).T,               # q: cq, sq
        (cos * rscale).T, (sin * rscale).T,             # k: ck, sk
    ], axis=1).astype(np.float32)                       # [32, 4, S]
    return np.ascontiguousarray(tabs)


def _host_consts():
    tabs = _host_tables()
    # diagonal-block causal mask: tri[r, j] = 1 if j >= r (valid q >= k)
    r = np.arange(128)[:, None]
    j = np.arange(128)[None, :]
    tri = (j >= r).astype(np.float32)                   # [128, 128]
    # rotate_half: out[i] = -in[16+i] (i<16); out[16+i] = in[i]
    P = np.zeros((NROT, NROT), dtype=np.float32)
    half = NROT // 2
    for i in range(half):
        P[i, half + i] = -1.0
        P[half + i, i] = 1.0
    pt = np.ascontiguousarray(P.T)
    ones = np.ones((128, 128), dtype=np.float32)
    return tabs, np.ascontiguousarray(tri), pt, ones


def _get_runner(repeat=1, bench=False):
    key = ("runner", repeat, bench)
    if key not in _CACHE:
        from runner_embedded import BassRunner
        nc = _build_program(repeat, bench=bench)
        _CACHE[key] = BassRunner(nc, n_cores=NCORES, donate=False)
    return _CACHE[key]


def make_in_maps(x, w_qkv, w_out):
    import ml_dtypes
    bf = ml_dtypes.bfloat16
    x = np.asarray(x, dtype=np.float32)
    w_qkv = np.asarray(w_qkv, dtype=np.float32)
    w_out = np.asarray(w_out, dtype=np.float32)
    xT = np.ascontiguousarray(x.reshape(S, D).T.astype(bf))
    tabs, tri, pt, ones = _host_consts()
    tabs, tri = tabs.astype(bf), tri.astype(bf)
    pt, ones = pt.astype(bf), ones.astype(bf)
    in_maps = []
    for c in range(NCORES):
        wslice = w_qkv[:, c * 128 * OCN:(c + 1) * 128 * OCN]
        # reference layout per head: [q(128)|k(128)|v(128)]; device layout:
        # [q0,k0,q1,k1,v0,v1]
        wq = np.concatenate([
            wslice[:, 0:128], wslice[:, 128:256],       # q0, k0
            wslice[:, 384:512], wslice[:, 512:640],     # q1, k1
            wslice[:, 256:384], wslice[:, 640:768],     # v0, v1
        ], axis=1)
        in_maps.append({
            "xT": xT,
            "wq": np.ascontiguousarray(wq.astype(bf)),
            "wo": np.ascontiguousarray(
                w_out[c * HPC * DQ:(c + 1) * HPC * DQ, :].astype(bf)),
            "tabs": tabs, "tri": tri, "pt": pt, "ones": ones,
        })
    return in_maps


def kernel(x, w_qkv, w_out):
    runner = _get_runner(repeat=1)
    in_maps = make_in_maps(x, w_qkv, w_out)
    results = runner(in_maps)
    y = np.zeros((S, D), dtype=np.float32)
    for c in range(NCORES):
        y += results[c]["y"].astype(np.float32)
    return y.reshape(1, S, D)


# ---------------------------------------------------------------------------
# Embedded PJRT runner (kernel.py must be self-contained).
import importlib.util as _ilu
import types as _types

_runner_src = '''
import sys
sys.path.insert(0, "/opt/trn_rl_repo")
import time
import numpy as np
import jax
import jax.numpy as jnp
from jax.experimental.shard_map import shard_map
from jax.sharding import Mesh, PartitionSpec

import concourse.mybir as mybir
from concourse.bass2jax import install_neuronx_cc_hook, _bass_exec_p, partition_id_tensor


class BassRunner:
    def __init__(self, nc, n_cores=8, donate=True):
        install_neuronx_cc_hook()
        self.nc = nc
        self.n_cores = n_cores
        self.donate = donate

        partition_name = nc.partition_id_tensor.name if nc.partition_id_tensor else None
        in_names, out_names, out_avals, zero_outs = [], [], [], []
        for alloc in nc.m.functions[0].allocations:
            if not isinstance(alloc, mybir.MemoryLocationSet):
                continue
            name = alloc.memorylocations[0].name
            if alloc.kind == "ExternalInput":
                if name != partition_name:
                    in_names.append(name)
            elif alloc.kind == "ExternalOutput":
                out_names.append(name)
                shape = tuple(alloc.tensor_shape)
                dtype = mybir.dt.np(alloc.dtype)
                out_avals.append(jax.core.ShapedArray(shape, dtype))
                zero_outs.append(np.zeros(shape, dtype))
        self.in_names = list(in_names)
        self.out_names = out_names
        self.out_avals = out_avals
        self.zero_outs = zero_outs
        n_params = len(in_names)
        n_outs = len(out_avals)
        all_in_names = list(in_names) + list(out_names)
        if partition_name is not None:
            all_in_names.append(partition_name)

        def _body(*args):
            operands = list(args)
            if partition_name is not None:
                operands.append(partition_id_tensor())
            outs = _bass_exec_p.bind(
                *operands,
                out_avals=tuple(out_avals),
                in_names=tuple(all_in_names),
                out_names=tuple(out_names),
                lowering_input_output_aliases=(),
                sim_require_finite=True,
                sim_require_nnan=True,
                nc=nc,
            )
            return tuple(outs)

        devices = jax.devices()[:n_cores]
        assert len(devices) == n_cores
        self.mesh = Mesh(np.asarray(devices), ("core",))
        in_specs = (PartitionSpec("core"),) * (n_params + n_outs)
        out_specs = (PartitionSpec("core"),) * n_outs
        donate_argnums = tuple(range(n_params, n_params + n_outs)) if donate else ()
        self.fn = jax.jit(
            shard_map(_body, mesh=self.mesh, in_specs=in_specs,
                      out_specs=out_specs, check_rep=False),
            donate_argnums=donate_argnums, keep_unused=True,
        )
        self.n_params = n_params
        self.n_outs = n_outs

    def concat_inputs(self, in_maps):
        return [
            np.concatenate([np.asarray(in_maps[c][name]) for c in range(self.n_cores)], axis=0)
            for name in self.in_names
        ]

    def __call__(self, in_maps):
        concat_in = self.concat_inputs(in_maps)
        concat_zeros = [
            np.zeros((self.n_cores * z.shape[0], *z.shape[1:]), z.dtype)
            for z in self.zero_outs
        ]
        out_arrs = self.fn(*concat_in, *concat_zeros)
        return [
            {name: np.asarray(out_arrs[i]).reshape(self.n_cores, *self.out_avals[i].shape)[c]
             for i, name in enumerate(self.out_names)}
            for c in range(self.n_cores)
        ]

    def sharded_inputs(self, in_maps):
        from jax.sharding import NamedSharding
        sh = NamedSharding(self.mesh, PartitionSpec("core"))
        concat_in = [jax.device_put(x, sh) for x in self.concat_inputs(in_maps)]
        concat_zeros = [
            jax.device_put(np.zeros((self.n_cores * z.shape[0], *z.shape[1:]), z.dtype), sh)
            for z in self.zero_outs
        ]
        return concat_in, concat_zeros

    def bench(self, in_maps, reps=10, warmup=2):
        assert not self.donate
        concat_in, concat_zeros = self.sharded_inputs(in_maps)
        times = []
        for i in range(reps + warmup):
            t0 = time.perf_counter()
            out = self.fn(*concat_in, *concat_zeros)
            jax.block_until_ready(out)
            dt = time.perf_counter() - t0
            if i >= warmup:
                times.append(dt)
        return times

    def bench_pipelined(self, in_maps, batch=20, warmup=3):
        assert not self.donate
        concat_in, concat_zeros = self.sharded_inputs(in_maps)
        for _ in range(warmup):
            jax.block_until_ready(self.fn(*concat_in, *concat_zeros))
        outs = []
        t0 = time.perf_counter()
        for _ in range(batch):
            outs.append(self.fn(*concat_in, *concat_zeros))
        jax.block_until_ready(outs)
        return (time.perf_counter() - t0) / batch
'''

_spec = _ilu.spec_from_loader("runner_embedded", loader=None)
_mod = _types.ModuleType("runner_embedded")
exec(_runner_src, _mod.__dict__)
sys.modules["runner_embedded"] = _mod


# revision 49
# speedup vs baseline: 1.0504x; 1.0504x over previous
"""Trainium2 Bass kernel for causal attention with xpos rotary embedding.

Reference computation (B=1, S=2048, D=2048, H=16 heads, dq=128):
    qkv = x @ w_qkv ; split into per-head q,k,v
    rope(q, scale), rope(k, 1/scale)  (xpos, first 32 dims of each head)
    causal softmax attention per head
    out = concat_heads @ w_out

Sharding: tensor-parallel over heads — each of the 8 cores gets 2 heads
(w_qkv column slice, w_out row slice), computes a full [S, D] partial of
the output projection; host sums the 8 partials (the "all-reduce").

Device kernel strategy (per core, bf16 data / f32 psum accumulate):
  One flat Tile region; phases interleaved per chunk so the scheduler can
  overlap PE (matmuls), ACT (exp), DVE (drains), Pool (rope), DMA.
  Phase A chunk c (CK=256 cols of x^T):
    qT/kT = w^T @ x^T per head (PE, bf16), rope applied in-place on the
    first 32 rows: rotate-half via host permutation matmul (PE) then
    3 tensor_tensor ops on the Pool engine (host cos/sin tables).
    v computed directly in [S, dq] layout (x-tile stationary) — no
    transposes.
  Phase B chunk (QC=512 q cols, after A covers its key span): per head,
  per ki pair: scoresT = kT_tile.T @ qT (PE) -> wide exp (ACT, sm_scale
  folded) -> diag-block mask (Pool) -> PV + ones-denominator accumulate
  (PE). Denominator reciprocal via DVE reciprocal_approx_fast; the
  normalize is folded into the PV psum->SBUF drain (DVE).
  Phase C (per q-tile, right after its B chunk): y = sum_h OT_h^T @ w_out
  (PE) -> psum drained to bf16 SBUF (DVE/ACT alternating) -> one DMA per
  q-tile row block. Host sums 8 bf16 partials in f32.
"""
import sys
sys.path.insert(0, "/opt/trn_rl_repo")
import numpy as np

S = 2048
D = 2048
NH = 16
DQ = 128
NROT = 32
SCALE_BASE = 512.0
NCORES = 8
HPC = NH // NCORES          # heads per core = 2
OCN = 3 * HPC               # 6 col tiles of 128 per core (q0,k0,q1,k1,v0,v1)
CK = 256                    # phase-A S-chunk width
NCK = S // CK
QC = 512                    # phase-B q-chunk width
NQC = S // QC
ST = S // 128               # 16 tiles
DC = 512                    # phase-C d-chunk width
SM_SCALE = float(1.0 / np.sqrt(DQ))

_CACHE = {}


def _build_program(repeat=1, bench=False):
    import concourse.bacc as bacc
    import concourse.tile as tile
    import concourse.mybir as mybir

    f32 = mybir.dt.float32
    bf16 = mybir.dt.bfloat16
    AF = mybir.ActivationFunctionType
    OP = mybir.AluOpType

    nc = bacc.Bacc("TRN2", target_bir_lowering=False, debug=False,
                   enable_asserts=False, num_devices=NCORES)

    xT_d = nc.dram_tensor("xT", [D, S], bf16, kind="ExternalInput").ap()
    wq_d = nc.dram_tensor("wq", [D, 128 * OCN], bf16, kind="ExternalInput").ap()
    wo_d = nc.dram_tensor("wo", [HPC * DQ, D], bf16, kind="ExternalInput").ap()
    tabs_d = nc.dram_tensor("tabs", [NROT, 4, S], bf16, kind="ExternalInput").ap()
    tri_d = nc.dram_tensor("tri", [128, 128], bf16, kind="ExternalInput").ap()
    pt_d = nc.dram_tensor("pt", [NROT, NROT], bf16, kind="ExternalInput").ap()
    ones_d = nc.dram_tensor("ones", [128, 128], bf16, kind="ExternalInput").ap()
    if bench:
        y_d = nc.dram_tensor("y", [S, D], bf16, kind="Internal").ap()
        ysm_d = nc.dram_tensor("ysm", [128, 64], bf16, kind="ExternalOutput").ap()
    else:
        y_d = nc.dram_tensor("y", [S, D], bf16, kind="ExternalOutput").ap()

    xT_r = xT_d.rearrange("(t p) s -> p t s", p=128)
    wq_r = wq_d.rearrange("(t p) f -> p t f", p=128)

    with tile.TileContext(nc) as tc:
      for rep in range(repeat):
        with tc.tile_pool(name="consts", bufs=1) as consts, \
             tc.tile_pool(name="persist", bufs=1) as persist, \
             tc.tile_pool(name="xtp", bufs=2) as xtp, \
             tc.tile_pool(name="rotp", bufs=2) as rotp, \
             tc.tile_pool(name="pp", bufs=3) as pp, \
             tc.tile_pool(name="recp", bufs=2) as recp, \
             tc.tile_pool(name="ysp", bufs=2) as ysp, \
             tc.tile_pool(name="psA", bufs=2, space="PSUM") as psA, \
             tc.tile_pool(name="psS", bufs=2, space="PSUM") as psS, \
             tc.tile_pool(name="psOT", bufs=1, space="PSUM") as psOT, \
             tc.tile_pool(name="psDen", bufs=1, space="PSUM") as psDen:

            # ---------------- input loads
            xt0 = xtp.tile([128, ST, CK], bf16, tag="xt")
            nc.sync.dma_start(xt0, xT_r[:, :, 0:CK])
            wq_sb = persist.tile([128, ST, 128 * OCN], bf16, tag="wq_sb")
            for t in range(ST):
                nc.sync.dma_start(wq_sb[:, t, :], wq_r[:, t, :])
            tabs = consts.tile([NROT, 4, S], bf16, tag="tabs")
            nc.sync.dma_start(tabs, tabs_d)
            pt = consts.tile([NROT, NROT], bf16, tag="pt")
            nc.sync.dma_start(pt, pt_d)
            tri = consts.tile([128, 128], bf16, tag="tri")
            nc.sync.dma_start(tri, tri_d)
            ones = consts.tile([128, 128], bf16, tag="ones")
            nc.sync.dma_start(ones, ones_d)
            wo_sb = persist.tile([128, HPC, D], bf16, tag="wo_sb")
            nc.sync.dma_start(wo_sb, wo_d.rearrange("(h p) f -> p h f", p=128))

            qT = [persist.tile([128, S], bf16, name=f"qT{h}", tag=f"qT{h}") for h in range(HPC)]
            kT = [persist.tile([128, S], bf16, name=f"kT{h}", tag=f"kT{h}") for h in range(HPC)]
            vn = [persist.tile([128, ST, 128], bf16, name=f"vn{h}", tag=f"vn{h}") for h in range(HPC)]
            OT = [persist.tile([128, S], bf16, name=f"OT{h}", tag=f"OT{h}") for h in range(HPC)]

            def emit_A_chunk(c, xt):
                cs = slice(c * CK, (c + 1) * CK)
                for oc in range(4):     # q_h0, k_h0, q_h1, k_h1
                    h, kind = divmod(oc, 2)
                    ps = psA.tile([128, CK], f32, tag="qkv")
                    for t in range(ST):
                        nc.tensor.matmul(
                            ps, wq_sb[:, t, oc * 128:(oc + 1) * 128],
                            xt[:, t, :], start=(t == 0), stop=(t == ST - 1))
                    dst = qT[h] if kind == 0 else kT[h]
                    if kind == 0:
                        nc.vector.tensor_copy(dst[:, cs], ps)
                    else:
                        nc.scalar.copy(dst[:, cs], ps)
                    # rope on rows 0:NROT, in place (rot psum rides the psA ring)
                    rps = psA.tile([128, CK], f32, tag="qkv")
                    nc.tensor.matmul(rps[0:NROT, :], pt, dst[0:NROT, cs],
                                     start=True, stop=True)
                    rot = rotp.tile([NROT, CK], bf16, tag="rotsb")
                    nc.scalar.copy(rot, rps[0:NROT, :])
                    ti = 0 if kind == 0 else 2
                    nc.gpsimd.tensor_tensor(
                        out=dst[0:NROT, cs], in0=dst[0:NROT, cs],
                        in1=tabs[:, ti, cs], op=OP.mult)
                    nc.gpsimd.tensor_tensor(
                        out=rot, in0=rot, in1=tabs[:, ti + 1, cs], op=OP.mult)
                    nc.gpsimd.tensor_tensor(
                        out=dst[0:NROT, cs], in0=dst[0:NROT, cs],
                        in1=rot, op=OP.add)
                # v directly in [s, dq] layout, both heads' cols at once
                # (shares the psA ring: same shape/tag as the qkv psum tiles)
                for si in range(CK // 128):
                    st = c * (CK // 128) + si
                    pv = psA.tile([128, 2 * 128], f32, tag="qkv")
                    for t in range(ST):
                        nc.tensor.matmul(
                            pv, xt[:, t, si * 128:(si + 1) * 128],
                            wq_sb[:, t, 4 * 128:6 * 128],
                            start=(t == 0), stop=(t == ST - 1))
                    nc.vector.tensor_copy(vn[0][:, st, :], pv[:, 0:128])
                    nc.scalar.copy(vn[1][:, st, :], pv[:, 128:256])

            def emit_B_chunk(qt0, qt1):
                # attention for q-tiles [qt0, qt1): history keys ki < qt0 at
                # full width, diagonal keys ki in [qt0, qt1) on the live
                # q-subrange only (triangle mask on the 128-wide diag block).
                w = 128 * (qt1 - qt0)
                q0 = 128 * qt0
                for h in range(HPC):
                    ot = psOT.tile([128, QC], f32, tag="ot")
                    dn = psDen.tile([128, QC], f32, tag="dn")
                    for m in range(qt1 - qt0):
                        ki = qt0 + m
                        qoff = 128 * m
                        sp = psS.tile([128, 2, QC], f32, tag="sc")
                        nc.tensor.matmul(
                            sp[:, 0, qoff:w], kT[h][:, ki * 128:(ki + 1) * 128],
                            qT[h][:, q0 + qoff:q0 + w], start=True, stop=True)
                        p = pp.tile([128, 2, QC], bf16, tag="p")
                        nc.scalar.activation(p[:, 0, qoff:w], sp[:, 0, qoff:w],
                                             AF.Exp, scale=SM_SCALE)
                        nc.vector.tensor_tensor(
                            out=p[:, 0, qoff:qoff + 128],
                            in0=p[:, 0, qoff:qoff + 128], in1=tri, op=OP.mult)
                        last = (qt0 == 0 and m == qt1 - qt0 - 1)
                        nc.tensor.matmul(
                            ot[:, qoff:w], vn[h][:, ki, :], p[:, 0, qoff:w],
                            start=(m == 0), stop=last, skip_group_check=True)
                        nc.tensor.matmul(
                            dn[:, qoff:w], ones, p[:, 0, qoff:w],
                            start=(m == 0), stop=last, skip_group_check=True)
                    # full-width history tiles (ki < qt0), paired exp
                    for kj in range(qt0 // 2):
                        sp = psS.tile([128, 2, QC], f32, tag="sc")
                        for u in range(2):
                            ki = 2 * kj + u
                            nc.tensor.matmul(
                                sp[:, u, 0:w], kT[h][:, ki * 128:(ki + 1) * 128],
                                qT[h][:, q0:q0 + w], start=True, stop=True)
                        p = pp.tile([128, 2, QC], bf16, tag="p")
                        nc.scalar.activation(p[:, :, 0:w], sp[:, :, 0:w],
                                             AF.Exp, scale=SM_SCALE)
                        last = (2 * kj + 1 == qt0 - 1)
                        for u in range(2):
                            nc.tensor.matmul(
                                ot[:, 0:w], vn[h][:, 2 * kj + u, :], p[:, u, 0:w],
                                start=False, stop=(last and u == 1),
                                skip_group_check=True)
                            nc.tensor.matmul(
                                dn[:, 0:w], ones, p[:, u, 0:w],
                                start=False, stop=(last and u == 1),
                                skip_group_check=True)
                    rec = recp.tile([128, QC], f32, tag="rec")
                    nc.vector.reciprocal_approx_fast(rec[:, 0:w], dn[:, 0:w])
                    nc.vector.tensor_tensor(
                        out=OT[h][:, q0:q0 + w], in0=ot[:, 0:w],
                        in1=rec[:, 0:w], op=OP.mult)

            def emit_C_chunk(qt0, qt1):
                # yp psum rides the psS ring ([128, 2, QC] = two DC-wide chunks)
                for qt in range(qt0, qt1):
                    ys = ysp.tile([128, D], bf16, tag="ys")
                    for dc2 in range(D // (2 * DC)):
                        yp = psS.tile([128, 2, QC], f32, tag="sc")
                        for u in range(2):
                            ds_ = slice((2 * dc2 + u) * DC, (2 * dc2 + u + 1) * DC)
                            for h in range(HPC):
                                nc.tensor.matmul(
                                    yp[:, u, :], OT[h][:, qt * 128:(qt + 1) * 128],
                                    wo_sb[:, h, ds_],
                                    start=(h == 0), stop=(h == HPC - 1))
                            if (dc2 + u) % 2 == 0:
                                nc.vector.tensor_copy(ys[:, ds_], yp[:, u, :])
                            else:
                                nc.scalar.copy(ys[:, ds_], yp[:, u, :])
                    nc.sync.dma_start(y_d[qt * 128:(qt + 1) * 128, :], ys)
                    if bench and qt == ST - 1:
                        nc.sync.dma_start(ysm_d, ys[:, 0:64])

            # B/C sub-chunk (qt0, qt1) emitted after A chunk (qt1 // 2) - 1;
            # the final 512-wide chunk is split so most of its work can
            # start before the last A chunk lands.
            bc_after = {1: [(0, 4)], 3: [(4, 8)], 5: [(8, 12)], 7: [(12, 16)]}
            for c in range(NCK):
                if c == 0:
                    xt = xt0
                else:
                    xt = xtp.tile([128, ST, CK], bf16, tag="xt")
                    nc.sync.dma_start(xt, xT_r[:, :, c * CK:(c + 1) * CK])
                emit_A_chunk(c, xt)
                for qt0, qt1 in bc_after.get(c, []):
                    emit_B_chunk(qt0, qt1)
                    emit_C_chunk(qt0, qt1)
    nc.compile()
    return nc


def _host_tables():
    """cos/sin xpos tables, computed in fp32 mirroring the jax reference."""
    t = np.arange(S, dtype=np.float32)
    inv_freq = (1.0 / (10000.0 ** (np.arange(0, NROT, 2, dtype=np.float32) / NROT))
                ).astype(np.float32)
    freqs = t[:, None] * inv_freq[None, :]              # [S, 16]
    pos = np.concatenate([freqs, freqs], axis=-1)       # [S, 32]
    base_scale = ((np.arange(0, NROT, 2, dtype=np.float32) + 0.4 * NROT)
                  / (1.4 * NROT)).astype(np.float32)
    power = (t - S // 2) / np.float32(SCALE_BASE)
    scale = base_scale[None, :] ** power[:, None]       # [S, 16]
    scale = np.concatenate([scale, scale], axis=-1)     # [S, 32]
    cos, sin = np.cos(pos), np.sin(pos)
    rscale = (np.float32(1.0) / scale).astype(np.float32)
    tabs = np.stack([
        (cos * scale).T, (sin * scale).T,               # q: cq, sq
        (cos * rscale).T, (sin * rscale).T,             # k: ck, sk
    ], axis=1).astype(np.float32)                       # [32, 4, S]
    return np.ascontiguousarray(tabs)


def _host_consts():
    tabs = _host_tables()
    # diagonal-block causal mask: tri[r, j] = 1 if j >= r (valid q >= k)
    r = np.arange(128)[:, None]
    j = np.arange(128)[None, :]
    tri = (j >= r).astype(np.float32)                   # [128, 128]
    # rotate_half: out[i] = -in[16+i] (i<16); out[16+i] = in[i]
    P = np.zeros((NROT, NROT), dtype=np.float32)
    half = NROT // 2
    for i in range(half):
        P[i, half + i] = -1.0
        P[half + i, i] = 1.0
    pt = np.ascontiguousarray(P.T)
    ones = np.ones((128, 128), dtype=np.float32)
    return tabs, np.ascontiguousarray(tri), pt, ones


def _get_runner(repeat=1, bench=False):
    key = ("runner", repeat, bench)
    if key not in _CACHE:
        from runner_embedded import BassRunner
        nc = _build_program(repeat, bench=bench)
        _CACHE[key] = BassRunner(nc, n_cores=NCORES, donate=False)
    return _CACHE[key]


def make_in_maps(x, w_qkv, w_out):
    import ml_dtypes
    bf = ml_dtypes.bfloat16
    x = np.asarray(x, dtype=np.float32)
    w_qkv = np.asarray(w_qkv, dtype=np.float32)
    w_out = np.asarray(w_out, dtype=np.float32)
    xT = np.ascontiguousarray(x.reshape(S, D).T.astype(bf))
    tabs, tri, pt, ones = _host_consts()
    tabs, tri = tabs.astype(bf), tri.astype(bf)
    pt, ones = pt.astype(bf), ones.astype(bf)
    in_maps = []
    for c in range(NCORES):
        wslice = w_qkv[:, c * 128 * OCN:(c + 1) * 128 * OCN]
        # reference layout per head: [q(128)|k(128)|v(128)]; device layout:
        # [q0,k0,q1,k1,v0,v1]
        wq = np.concatenate([
            wslice[:, 0:128], wslice[:, 128:256],       # q0, k0
            wslice[:, 384:512], wslice[:, 512:640],     # q1, k1
            wslice[:, 256:384], wslice[:, 640:768],     # v0, v1
        ], axis=1)
        in_maps.append({
            "xT": xT,
            "wq": np.ascontiguousarray(wq.astype(bf)),
            "wo": np.ascontiguousarray(
                w_out[c * HPC * DQ:(c + 1) * HPC * DQ, :].astype(bf)),
            "tabs": tabs, "tri": tri, "pt": pt, "ones": ones,
        })
    return in_maps


def kernel(x, w_qkv, w_out):
    runner = _get_runner(repeat=1)
    in_maps = make_in_maps(x, w_qkv, w_out)
    results = runner(in_maps)
    y = np.zeros((S, D), dtype=np.float32)
    for c in range(NCORES):
        y += results[c]["y"].astype(np.float32)
    return y.reshape(1, S, D)


# ---------------------------------------------------------------------------
# Embedded PJRT runner (kernel.py must be self-contained).
import importlib.util as _ilu
import types as _types

_runner_src = '''
import sys
sys.path.insert(0, "/opt/trn_rl_repo")
import time
import numpy as np
import jax
import jax.numpy as jnp
from jax.experimental.shard_map import shard_map
from jax.sharding import Mesh, PartitionSpec

import concourse.mybir as mybir
from concourse.bass2jax import install_neuronx_cc_hook, _bass_exec_p, partition_id_tensor


class BassRunner:
    def __init__(self, nc, n_cores=8, donate=True):
        install_neuronx_cc_hook()
        self.nc = nc
        self.n_cores = n_cores
        self.donate = donate

        partition_name = nc.partition_id_tensor.name if nc.partition_id_tensor else None
        in_names, out_names, out_avals, zero_outs = [], [], [], []
        for alloc in nc.m.functions[0].allocations:
            if not isinstance(alloc, mybir.MemoryLocationSet):
                continue
            name = alloc.memorylocations[0].name
            if alloc.kind == "ExternalInput":
                if name != partition_name:
                    in_names.append(name)
            elif alloc.kind == "ExternalOutput":
                out_names.append(name)
                shape = tuple(alloc.tensor_shape)
                dtype = mybir.dt.np(alloc.dtype)
                out_avals.append(jax.core.ShapedArray(shape, dtype))
                zero_outs.append(np.zeros(shape, dtype))
        self.in_names = list(in_names)
        self.out_names = out_names
        self.out_avals = out_avals
        self.zero_outs = zero_outs
        n_params = len(in_names)
        n_outs = len(out_avals)
        all_in_names = list(in_names) + list(out_names)
        if partition_name is not None:
            all_in_names.append(partition_name)

        def _body(*args):
            operands = list(args)
            if partition_name is not None:
                operands.append(partition_id_tensor())
            outs = _bass_exec_p.bind(
                *operands,
                out_avals=tuple(out_avals),
                in_names=tuple(all_in_names),
                out_names=tuple(out_names),
                lowering_input_output_aliases=(),
                sim_require_finite=True,
                sim_require_nnan=True,
                nc=nc,
            )
            return tuple(outs)

        devices = jax.devices()[:n_cores]
        assert len(devices) == n_cores
        self.mesh = Mesh(np.asarray(devices), ("core",))
        in_specs = (PartitionSpec("core"),) * (n_params + n_outs)
        out_specs = (PartitionSpec("core"),) * n_outs
        donate_argnums = tuple(range(n_params, n_params + n_outs)) if donate else ()
        self.fn = jax.jit(
            shard_map(_body, mesh=self.mesh, in_specs=in_specs,
                      out_specs=out_specs, check_rep=False),
            donate_argnums=donate_argnums, keep_unused=True,
        )
        self.n_params = n_params
        self.n_outs = n_outs

    def concat_inputs(self, in_maps):
        return [
            np.concatenate([np.asarray(in_maps[c][name]) for c in range(self.n_cores)], axis=0)
            for name in self.in_names
        ]

    def __call__(self, in_maps):
        concat_in = self.concat_inputs(in_maps)
        concat_zeros = [
            np.zeros((self.n_cores * z.shape[0], *z.shape[1:]), z.dtype)
            for z in self.zero_outs
        ]
        out_arrs = self.fn(*concat_in, *concat_zeros)
        return [
            {name: np.asarray(out_arrs[i]).reshape(self.n_cores, *self.out_avals[i].shape)[c]
             for i, name in enumerate(self.out_names)}
            for c in range(self.n_cores)
        ]

    def sharded_inputs(self, in_maps):
        from jax.sharding import NamedSharding
        sh = NamedSharding(self.mesh, PartitionSpec("core"))
        concat_in = [jax.device_put(x, sh) for x in self.concat_inputs(in_maps)]
        concat_zeros = [
            jax.device_put(np.zeros((self.n_cores * z.shape[0], *z.shape[1:]), z.dtype), sh)
            for z in self.zero_outs
        ]
        return concat_in, concat_zeros

    def bench(self, in_maps, reps=10, warmup=2):
        assert not self.donate
        concat_in, concat_zeros = self.sharded_inputs(in_maps)
        times = []
        for i in range(reps + warmup):
            t0 = time.perf_counter()
            out = self.fn(*concat_in, *concat_zeros)
            jax.block_until_ready(out)
            dt = time.perf_counter() - t0
            if i >= warmup:
                times.append(dt)
        return times

    def bench_pipelined(self, in_maps, batch=20, warmup=3):
        assert not self.donate
        concat_in, concat_zeros = self.sharded_inputs(in_maps)
        for _ in range(warmup):
            jax.block_until_ready(self.fn(*concat_in, *concat_zeros))
        outs = []
        t0 = time.perf_counter()
        for _ in range(batch):
            outs.append(self.fn(*concat_in, *concat_zeros))
        jax.block_until_ready(outs)
        return (time.perf_counter() - t0) / batch
'''

_spec = _ilu.spec_from_loader("runner_embedded", loader=None)
_mod = _types.ModuleType("runner_embedded")
exec(_runner_src, _mod.__dict__)
sys.modules["runner_embedded"] = _mod
